# revision 3
# baseline (speedup 1.0000x reference)
"""Multi-head attention (B=2, N=4096, C=512, H=8, D=64) on 8 TRN2 NeuronCores.

Sharding: data-parallel over batch (2 groups of 4 cores) x tensor-parallel over
heads (2 heads/core). Per core: qkv projection, attention for its 2 heads, and
a partial output projection y_partial^T = Wp_slice^T @ attn^T; the host sums
the 4 per-batch partials, transposes, adds bias.

Engine layout (v2 — restructured for overlap):
- x is loaded in 512-col slices; per slice the K/V projections run and the
  first attention group's tiles interleave, so ScalarE exp starts at ~6us
  instead of ~67us. Q blocks (except block 0) are projected lazily one group
  ahead of their use.
- All PSUM attention traffic lives in one [128, 3072] ring (6 banks, 512-col
  halves) plus a 2-buf [128,512] pool for PV accumulation / proj. A greedy
  allocator hands out ring halves; when two QK slots land contiguously the
  exp runs as one 2048-col ACTIVATE, halving ScalarE instruction overhead.
- exp(S) on ScalarE from PSUM; softmax denominator rides the PV matmul as a
  ones-column appended to V (lhsT is [V_h | 1], M=65); the denominator row is
  moved to partition 0 by a small DMA, inverted on VectorE, and
  partition-broadcast on GpSimd.
"""
import os
import sys

for _p in ("/opt/trn_rl_repo", "/root/.axon_site/_ro/trn_rl_repo"):
    if os.path.isdir(_p) and _p not in sys.path:
        sys.path.append(_p)

import numpy as np
from contextlib import ExitStack

import concourse.bass as bass
import concourse.mybir as mybir
import concourse.tile as tile
from concourse import bacc
from concourse.bass_utils import run_bass_kernel_spmd

F32 = mybir.dt.float32
BF16 = mybir.dt.bfloat16
EXP = mybir.ActivationFunctionType.Exp

DIM, N, HD = 512, 4096, 64
SCALE = HD ** -0.5
NB = N // 512    # 8  n-blocks of 512 queries
MB = N // 128    # 32 m-chunks of 128 keys
CC = DIM // 128  # 4  c-chunks of the model dim
SLOTS = 16       # 1024-col slots (2 m-chunks) per (nb, h) group
RH = 6           # ring halves (512 cols / 1 PSUM bank each)


def build_nc():
    nc = bacc.Bacc("TRN2", target_bir_lowering=False)
    xT = nc.declare_dram_parameter("xT", [DIM, N], F32, isOutput=False)
    wqkvT = nc.declare_dram_parameter("wqkvT", [DIM, 384], F32, isOutput=False)
    wpT = nc.declare_dram_parameter("wpT", [128, DIM], F32, isOutput=False)
    out = nc.declare_dram_parameter("out", [DIM, N], F32, isOutput=True)

    with ExitStack() as ctx:
        tc = ctx.enter_context(tile.TileContext(nc))
        big = ctx.enter_context(tc.tile_pool(name="big", bufs=1))
        stage = ctx.enter_context(tc.tile_pool(name="stage", bufs=8))
        esp = ctx.enter_context(tc.tile_pool(name="esp", bufs=3))
        yup = ctx.enter_context(tc.tile_pool(name="yup", bufs=2))
        ysp = ctx.enter_context(tc.tile_pool(name="ysp", bufs=3))
        ring_p = ctx.enter_context(tc.tile_pool(name="ring", bufs=1, space="PSUM"))
        po_p = ctx.enter_context(tc.tile_pool(name="psB", bufs=2, space="PSUM"))

        # ---- persistent SBUF ----
        wq = [big.tile([128, 384], BF16, tag=f"wqb{c}", name=f"wqb{c}") for c in range(CC)]
        wpb = big.tile([128, DIM], BF16, tag="wpb", name="wpb")
        xtb = [big.tile([128, N], BF16, tag=f"xtb{c}", name=f"xtb{c}") for c in range(CC)]
        qt = big.tile([128, N], BF16, tag="qt", name="qt")
        kt = [big.tile([128, N], BF16, tag=f"kt{h}", name=f"kt{h}") for h in range(2)]
        v2 = big.tile([128, 130 * MB], BF16, tag="v2", name="v2")
        atB = big.tile([128, N], BF16, tag="atB", name="atB")
        ring = ring_p.tile([128, RH * 512], F32, tag="ring", name="ring")

        # warm the exp table set while DMAs are in flight
        dummy = big.tile([1, 8], F32, tag="dummy", name="dummy")
        nc.vector.memset(dummy[:], 0.0)
        nc.scalar.activation(out=dummy[:], in_=dummy[:], func=EXP, scale=1.0)

        # ones columns of the [V_h0 | 1 | V_h1 | 1] layout (strided; V copies
        # fill the rest, so no whole-tile memset serializing against them)
        nc.vector.memset(v2[:, 64::130], 1.0)
        nc.vector.memset(v2[:, 129::130], 1.0)
        # per-head K zero padding (other head's partitions)
        nc.gpsimd.memset(kt[0][64:128, :], 0.0)
        nc.gpsimd.memset(kt[1][0:64, :], 0.0)

        # ---- weight loads (vector casts: cheap, and gpsimd is busy zeroing) ----
        for cc in range(CC):
            t = stage.tile([128, 384], F32, tag="wst", name="wst")
            nc.sync.dma_start(out=t[:], in_=wqkvT[cc * 128:(cc + 1) * 128, :])
            nc.vector.tensor_copy(out=wq[cc][:], in_=t[:])
        t = stage.tile([128, DIM], F32, tag="wpst", name="wpst")
        nc.sync.dma_start(out=t[:], in_=wpT[:, :])
        nc.gpsimd.tensor_copy(out=wpb[:], in_=t[:])

        # ---- ring allocator (units of 512-col halves) ----
        rp = [0]

        def take(n):
            if rp[0] + n > RH:
                rp[0] = 0
            start = rp[0]
            rp[0] = (rp[0] + n) % RH
            return start

        # ---- emit helpers ----
        def emit_q(nb):
            ns = slice(nb * 512, (nb + 1) * 512)
            half = take(1)
            ps = ring[:, half * 512:(half + 1) * 512]
            for cc in range(CC):
                nc.tensor.matmul(
                    ps, lhsT=wq[cc][:, 0:128], rhs=xtb[cc][:, ns],
                    start=(cc == 0), stop=(cc == CC - 1),
                )
            nc.vector.tensor_copy(out=qt[:, ns], in_=ps)

        def emit_k(nb):
            ns = slice(nb * 512, (nb + 1) * 512)
            half = take(1)
            ps = ring[:, half * 512:(half + 1) * 512]
            for cc in range(CC):
                nc.tensor.matmul(
                    ps, lhsT=wq[cc][:, 128:256], rhs=xtb[cc][:, ns],
                    start=(cc == 0), stop=(cc == CC - 1),
                )
            nc.vector.tensor_copy(out=kt[0][0:64, ns], in_=ps[0:64, :])
            nc.vector.tensor_copy(out=kt[1][64:128, ns], in_=ps[64:128, :])

        def emit_v4(s):
            # V for m-chunks 4s..4s+3, key-partitioned, packed in one ring half
            half = take(1)
            for k in range(4):
                mb = 4 * s + k
                ps = ring[:, half * 512 + k * 128:half * 512 + (k + 1) * 128]
                for cc in range(CC):
                    nc.tensor.matmul(
                        ps, lhsT=xtb[cc][:, mb * 128:(mb + 1) * 128],
                        rhs=wq[cc][:, 256:384],
                        start=(cc == 0), stop=(cc == CC - 1),
                    )
            for k in range(4):
                mb = 4 * s + k
                base = half * 512 + k * 128
                nc.vector.tensor_copy(out=v2[:, mb * 130:mb * 130 + 64], in_=ring[:, base:base + 64])
                nc.vector.tensor_copy(out=v2[:, mb * 130 + 65:mb * 130 + 129], in_=ring[:, base + 64:base + 128])

        def emit_proj(nb):
            ns = slice(nb * 512, (nb + 1) * 512)
            for ob in range(4):
                pp = po_p.tile([128, 512], F32, tag="po", name="pp")
                nc.tensor.matmul(
                    pp[:], lhsT=wpb[:, ob * 128:(ob + 1) * 128], rhs=atB[:, ns],
                    start=True, stop=True,
                )
                ys = ysp.tile([128, 512], F32, tag="ys", name="ys")
                nc.vector.tensor_copy(out=ys[:], in_=pp[:])
                nc.sync.dma_start(out=out[ob * 128:(ob + 1) * 128, ns], in_=ys[:])

        def emit_norm(nb, h, po):
            ns = slice(nb * 512, (nb + 1) * 512)
            yu = yup.tile([128, 512], F32, tag="yu", name="yu")
            nc.vector.tensor_copy(out=yu[0:65, :], in_=po[0:65, :])
            row = yup.tile([1, 512], F32, tag="row", name="row")
            nc.sync.dma_start(out=row[:], in_=yu[64:65, :])
            den = yup.tile([64, 512], F32, tag="den", name="den")
            nc.gpsimd.partition_broadcast(den[:], row[0:1, :])
            rec = yup.tile([64, 512], F32, tag="rec", name="rec")
            nc.vector.reciprocal_approx_fast(out=rec[:], in_=den[:])
            if h == 0:
                nc.vector.tensor_mul(out=atB[0:64, ns], in0=yu[0:64, :], in1=rec[:])
                if nb > 0:
                    emit_proj(nb - 1)
            else:
                a1 = yup.tile([64, 512], BF16, tag="a1", name="a1")
                nc.vector.tensor_mul(out=a1[:], in0=yu[0:64, :], in1=rec[:])
                nc.sync.dma_start(out=atB[64:128, ns], in_=a1[:])

        # pend: (nb, h, po, es_ap, [(mb, escol)...]) for the un-PV'd last unit
        pend = [None]

        def flush_pend():
            pnb, ph, ppo, pes, pchunks = pend[0]
            for mb, ec in pchunks:
                nc.tensor.matmul(
                    ppo[0:65, :],
                    lhsT=v2[:, mb * 130 + 65 * ph:mb * 130 + 65 * ph + 65],
                    rhs=pes[:, ec:ec + 512],
                    start=(mb == 0), stop=(mb == MB - 1),
                )
            emit_norm(pnb, ph, ppo)
            pend[0] = None

        class Group:
            """Attention (nb, h): QK into ring slots, exp units, PV lagging one
            unit; last unit's PV is deferred to pend."""

            def __init__(self, nb, h):
                self.nb, self.h = nb, h
                self.ns = slice(nb * 512, (nb + 1) * 512)
                self.po = po_p.tile([128, 512], F32, tag="po", name="po")
                self.done = 0          # slots emitted
                self.prev = None       # (es, [(mb, escol)...])
                self.first = True

            def emit_slots(self, upto):
                while self.done < upto:
                    pair = (upto - self.done >= 2) and (rp[0] + 4 <= RH)
                    w = 2 if pair else 1
                    start = take(2 * w)
                    chunks = []
                    for i in range(w):
                        t = self.done + i
                        for j in range(2):
                            mb = 2 * t + j
                            half = start + 2 * i + j
                            nc.tensor.matmul(
                                ring[:, half * 512:(half + 1) * 512],
                                lhsT=kt[self.h][:, mb * 128:(mb + 1) * 128],
                                rhs=qt[:, self.ns],
                                start=True, stop=True,
                            )
                            chunks.append((mb, (2 * i + j) * 512))
                    es = esp.tile([128, 1024 * w], BF16, tag=f"es{w}", name="es")
                    nc.scalar.activation(
                        out=es[:, 0:1024 * w],
                        in_=ring[:, start * 512:(start + 2 * w) * 512],
                        func=EXP, scale=SCALE,
                    )
                    if self.first and pend[0] is not None:
                        flush_pend()
                    self.first = False
                    if self.prev is not None:
                        pes, pchunks = self.prev
                        for mb, ec in pchunks:
                            nc.tensor.matmul(
                                self.po[0:65, :],
                                lhsT=v2[:, mb * 130 + 65 * self.h:mb * 130 + 65 * self.h + 65],
                                rhs=pes[:, ec:ec + 512],
                                start=(mb == 0), stop=(mb == MB - 1),
                            )
                    self.prev = (es, chunks)
                    self.done += w

            def finish(self):
                self.emit_slots(SLOTS)
                pend[0] = (self.nb, self.h, self.po, *self.prev)

        # ---- prologue: sliced x load, K/V proj, group (0,0) interleaved ----
        g00 = Group(0, 0)
        cast_eng = [nc.vector.tensor_copy, nc.vector.tensor_copy,
                    nc.scalar.copy, nc.gpsimd.tensor_copy]
        for s in range(NB):
            ns = slice(s * 512, (s + 1) * 512)
            stg = []
            for cc in range(CC):
                t = stage.tile([128, 512], F32, tag="xs", name="xs")
                nc.sync.dma_start(out=t[:], in_=xT[cc * 128:(cc + 1) * 128, ns])
                stg.append(t)
            for cc in range(CC):
                cast_eng[cc](out=xtb[cc][:, ns], in_=stg[cc][:])
            emit_k(s)
            if s == 0:
                emit_q(0)
            emit_v4(s)
            g00.emit_slots(min(2 * s + 1, SLOTS))
        g00.finish()

        # ---- steady state ----
        emit_q(1)
        Group(0, 1).finish()
        for nb in range(1, NB):
            Group(nb, 0).finish()
            if nb + 1 < NB:
                emit_q(nb + 1)
            Group(nb, 1).finish()
        flush_pend()
        emit_proj(NB - 1)

    nc.compile()
    return nc


_NC_CACHE = None
LAST_EXEC_NS = None


def kernel(x, w_qkv, w_proj, b_proj):
    global _NC_CACHE, LAST_EXEC_NS
    x = np.ascontiguousarray(np.asarray(x, dtype=np.float32))
    w_qkv = np.asarray(w_qkv, dtype=np.float32)
    w_proj = np.asarray(w_proj, dtype=np.float32)
    b_proj = np.asarray(b_proj, dtype=np.float32)
    B = x.shape[0]

    if _NC_CACHE is None:
        _NC_CACHE = build_nc()
    nc = _NC_CACHE

    xTs = [np.ascontiguousarray(x[b].T) for b in range(B)]
    in_maps = []
    for c in range(8):
        b, hp = c // 4, c % 4
        qr = w_qkv[2 * hp * 64:2 * hp * 64 + 128]
        kr = w_qkv[512 + 2 * hp * 64:512 + 2 * hp * 64 + 128]
        vr = w_qkv[1024 + 2 * hp * 64:1024 + 2 * hp * 64 + 128]
        wqkvT = np.ascontiguousarray(np.concatenate([qr, kr, vr], 0).T)
        wpT = np.ascontiguousarray(w_proj[:, hp * 128:(hp + 1) * 128].T)
        in_maps.append({"xT": xTs[b], "wqkvT": wqkvT, "wpT": wpT})

    res = run_bass_kernel_spmd(
        nc,
        in_maps,
        core_ids=list(range(8)),
        trace=bool(int(os.environ.get("ATTN_TRACE", "0"))),
    )
    LAST_EXEC_NS = res.exec_time_ns

    out = np.zeros((B, N, DIM), np.float32)
    for b in range(B):
        acc = res.results[4 * b]["out"].copy()
        for c in range(4 * b + 1, 4 * b + 4):
            acc += res.results[c]["out"]
        out[b] = acc.T + b_proj
    return out


# revision 7
# speedup vs baseline: 1.0032x; 1.0032x over previous
"""Multi-head attention (B=2, N=4096, C=512, H=8, D=64) on 8 TRN2 NeuronCores.

Sharding: data-parallel over batch (2 groups of 4 cores) x tensor-parallel over
heads (2 heads/core). Per core: qkv projection, attention for its 2 heads, and
a partial output projection y_partial^T = Wp_slice^T @ attn^T; the host sums
the 4 per-batch partials, transposes, adds bias.

Engine layout (v2 — restructured for overlap):
- x is loaded in 512-col slices; per slice the K/V projections run and the
  first attention group's tiles interleave, so ScalarE exp starts at ~6us
  instead of ~67us. Q blocks (except block 0) are projected lazily one group
  ahead of their use.
- All PSUM attention traffic lives in one [128, 3072] ring (6 banks, 512-col
  halves) plus a 2-buf [128,512] pool for PV accumulation / proj. A greedy
  allocator hands out ring halves; when two QK slots land contiguously the
  exp runs as one 2048-col ACTIVATE, halving ScalarE instruction overhead.
- exp(S) on ScalarE from PSUM; softmax denominator rides the PV matmul as a
  ones-column appended to V (lhsT is [V_h | 1], M=65); the denominator row is
  moved to partition 0 by a small DMA, inverted on VectorE, and
  partition-broadcast on GpSimd.
"""
import os
import sys

for _p in ("/opt/trn_rl_repo", "/root/.axon_site/_ro/trn_rl_repo"):
    if os.path.isdir(_p) and _p not in sys.path:
        sys.path.append(_p)

import numpy as np
from contextlib import ExitStack

import concourse.bass as bass
import concourse.mybir as mybir
import concourse.tile as tile
from concourse import bacc
from concourse.bass_utils import run_bass_kernel_spmd

F32 = mybir.dt.float32
BF16 = mybir.dt.bfloat16
EXP = mybir.ActivationFunctionType.Exp

DIM, N, HD = 512, 4096, 64
SCALE = HD ** -0.5
NB = N // 512    # 8  n-blocks of 512 queries
MB = N // 128    # 32 m-chunks of 128 keys
CC = DIM // 128  # 4  c-chunks of the model dim
SLOTS = 16       # 1024-col slots (2 m-chunks) per (nb, h) group
RH = 6           # ring halves (512 cols / 1 PSUM bank each)


def build_nc():
    nc = bacc.Bacc("TRN2", target_bir_lowering=False)
    xT = nc.declare_dram_parameter("xT", [DIM, N], F32, isOutput=False)
    wqkvT = nc.declare_dram_parameter("wqkvT", [DIM, 384], F32, isOutput=False)
    wpT = nc.declare_dram_parameter("wpT", [128, DIM], F32, isOutput=False)
    out = nc.declare_dram_parameter("out", [DIM, N], F32, isOutput=True)

    with ExitStack() as ctx:
        tc = ctx.enter_context(tile.TileContext(nc))
        big = ctx.enter_context(tc.tile_pool(name="big", bufs=1))
        stage = ctx.enter_context(tc.tile_pool(name="stage", bufs=8))
        esp = ctx.enter_context(tc.tile_pool(name="esp", bufs=3))
        yup = ctx.enter_context(tc.tile_pool(name="yup", bufs=2))
        ysp = ctx.enter_context(tc.tile_pool(name="ysp", bufs=3))
        ring_p = ctx.enter_context(tc.tile_pool(name="ring", bufs=1, space="PSUM"))
        po_p = ctx.enter_context(tc.tile_pool(name="psB", bufs=2, space="PSUM"))

        # ---- persistent SBUF ----
        wq = [big.tile([128, 384], BF16, tag=f"wqb{c}", name=f"wqb{c}") for c in range(CC)]
        wpb = big.tile([128, DIM], BF16, tag="wpb", name="wpb")
        xtb = [big.tile([128, N], BF16, tag=f"xtb{c}", name=f"xtb{c}") for c in range(CC)]
        qt = big.tile([128, N], BF16, tag="qt", name="qt")
        kt = [big.tile([128, N], BF16, tag=f"kt{h}", name=f"kt{h}") for h in range(2)]
        v2 = big.tile([128, 130 * MB], BF16, tag="v2", name="v2")
        atB = big.tile([128, N], BF16, tag="atB", name="atB")
        ring = ring_p.tile([128, RH * 512], F32, tag="ring", name="ring")

        # warm the exp table set while DMAs are in flight
        dummy = big.tile([1, 8], F32, tag="dummy", name="dummy")
        nc.vector.memset(dummy[:], 0.0)
        nc.scalar.activation(out=dummy[:], in_=dummy[:], func=EXP, scale=1.0)

        # ones columns of the [V_h0 | 1 | V_h1 | 1] layout (strided; V copies
        # fill the rest, so no whole-tile memset serializing against them)
        nc.vector.memset(v2[:, 64::130], 1.0)
        nc.vector.memset(v2[:, 129::130], 1.0)
        # per-head K zero padding (other head's partitions)
        nc.gpsimd.memset(kt[0][64:128, :], 0.0)
        nc.gpsimd.memset(kt[1][0:64, :], 0.0)

        # ---- weight loads (vector casts: cheap, and gpsimd is busy zeroing) ----
        for cc in range(CC):
            t = stage.tile([128, 384], F32, tag="wst", name="wst")
            nc.sync.dma_start(out=t[:], in_=wqkvT[cc * 128:(cc + 1) * 128, :])
            nc.vector.tensor_copy(out=wq[cc][:], in_=t[:])
        t = stage.tile([128, DIM], F32, tag="wpst", name="wpst")
        nc.sync.dma_start(out=t[:], in_=wpT[:, :])
        nc.gpsimd.tensor_copy(out=wpb[:], in_=t[:])

        # ---- ring allocator (units of 512-col halves) ----
        rp = [0]

        def take(n):
            if rp[0] + n > RH:
                rp[0] = 0
            start = rp[0]
            rp[0] = (rp[0] + n) % RH
            return start

        # ---- emit helpers ----
        def emit_q(nb):
            ns = slice(nb * 512, (nb + 1) * 512)
            half = take(1)
            ps = ring[:, half * 512:(half + 1) * 512]
            for cc in range(CC):
                nc.tensor.matmul(
                    ps, lhsT=wq[cc][:, 0:128], rhs=xtb[cc][:, ns],
                    start=(cc == 0), stop=(cc == CC - 1),
                )
            nc.vector.tensor_copy(out=qt[:, ns], in_=ps)

        def emit_k(nb):
            ns = slice(nb * 512, (nb + 1) * 512)
            half = take(1)
            ps = ring[:, half * 512:(half + 1) * 512]
            for cc in range(CC):
                nc.tensor.matmul(
                    ps, lhsT=wq[cc][:, 128:256], rhs=xtb[cc][:, ns],
                    start=(cc == 0), stop=(cc == CC - 1),
                )
            nc.vector.tensor_copy(out=kt[0][0:64, ns], in_=ps[0:64, :])
            nc.vector.tensor_copy(out=kt[1][64:128, ns], in_=ps[64:128, :])

        def emit_v4(s):
            # V for m-chunks 4s..4s+3, key-partitioned, packed in one ring half
            half = take(1)
            for k in range(4):
                mb = 4 * s + k
                ps = ring[:, half * 512 + k * 128:half * 512 + (k + 1) * 128]
                for cc in range(CC):
                    nc.tensor.matmul(
                        ps, lhsT=xtb[cc][:, mb * 128:(mb + 1) * 128],
                        rhs=wq[cc][:, 256:384],
                        start=(cc == 0), stop=(cc == CC - 1),
                    )
            for k in range(4):
                mb = 4 * s + k
                base = half * 512 + k * 128
                nc.vector.tensor_copy(out=v2[:, mb * 130:mb * 130 + 64], in_=ring[:, base:base + 64])
                nc.vector.tensor_copy(out=v2[:, mb * 130 + 65:mb * 130 + 129], in_=ring[:, base + 64:base + 128])

        def emit_proj(nb):
            ns = slice(nb * 512, (nb + 1) * 512)
            for ob in range(4):
                pp = po_p.tile([128, 512], F32, tag="po", name="pp")
                nc.tensor.matmul(
                    pp[:], lhsT=wpb[:, ob * 128:(ob + 1) * 128], rhs=atB[:, ns],
                    start=True, stop=True,
                )
                ys = ysp.tile([128, 512], F32, tag="ys", name="ys")
                nc.vector.tensor_copy(out=ys[:], in_=pp[:])
                nc.sync.dma_start(out=out[ob * 128:(ob + 1) * 128, ns], in_=ys[:])

        def emit_norm(nb, h, po):
            ns = slice(nb * 512, (nb + 1) * 512)
            yu = yup.tile([128, 512], F32, tag="yu", name="yu")
            nc.vector.tensor_copy(out=yu[0:65, :], in_=po[0:65, :])
            row = yup.tile([1, 512], F32, tag="row", name="row")
            nc.sync.dma_start(out=row[:], in_=yu[64:65, :])
            den = yup.tile([64, 512], F32, tag="den", name="den")
            nc.gpsimd.partition_broadcast(den[:], row[0:1, :])
            rec = yup.tile([64, 512], F32, tag="rec", name="rec")
            nc.vector.reciprocal_approx_fast(out=rec[:], in_=den[:])
            if h == 0:
                nc.vector.tensor_mul(out=atB[0:64, ns], in0=yu[0:64, :], in1=rec[:])
                if nb > 0:
                    emit_proj(nb - 1)
            else:
                a1 = yup.tile([64, 512], BF16, tag="a1", name="a1")
                nc.vector.tensor_mul(out=a1[:], in0=yu[0:64, :], in1=rec[:])
                nc.sync.dma_start(out=atB[64:128, ns], in_=a1[:])

        # pend: (nb, h, po, [(es_ap, [(mb, escol)...]), ...]) — the last two
        # un-PV'd units of the previous group (PV lags 2 units so an ACT's
        # completion never sits on the PE critical path)
        pend = [None]

        def flush_pend():
            pnb, ph, ppo, punits = pend[0]
            for pes, pchunks in punits:
                for mb, ec in pchunks:
                    nc.tensor.matmul(
                        ppo[0:65, :],
                        lhsT=v2[:, mb * 130 + 65 * ph:mb * 130 + 65 * ph + 65],
                        rhs=pes[:, ec:ec + 512],
                        start=(mb == 0), stop=(mb == MB - 1),
                    )
            emit_norm(pnb, ph, ppo)
            pend[0] = None

        class Group:
            """Attention (nb, h): QK into ring slots, exp units, PV lagging one
            unit; last unit's PV is deferred to pend."""

            def __init__(self, nb, h):
                self.nb, self.h = nb, h
                self.ns = slice(nb * 512, (nb + 1) * 512)
                self.po = po_p.tile([128, 512], F32, tag="po", name="po")
                self.done = 0          # slots emitted
                self.q = []            # un-PV'd units: (es, [(mb, escol)...])
                self.first = True

            def emit_slots(self, upto):
                while self.done < upto:
                    pair = (upto - self.done >= 2) and (rp[0] + 4 <= RH)
                    w = 2 if pair else 1
                    start = take(2 * w)
                    chunks = []
                    for i in range(w):
                        t = self.done + i
                        for j in range(2):
                            mb = 2 * t + j
                            half = start + 2 * i + j
                            nc.tensor.matmul(
                                ring[:, half * 512:(half + 1) * 512],
                                lhsT=kt[self.h][:, mb * 128:(mb + 1) * 128],
                                rhs=qt[:, self.ns],
                                start=True, stop=True,
                            )
                            chunks.append((mb, (2 * i + j) * 512))
                    es = esp.tile([128, 1024 * w], BF16, tag=f"es{w}", name="es")
                    nc.scalar.activation(
                        out=es[:, 0:1024 * w],
                        in_=ring[:, start * 512:(start + 2 * w) * 512],
                        func=EXP, scale=SCALE,
                    )
                    if self.first and pend[0] is not None:
                        flush_pend()
                    self.first = False
                    if len(self.q) >= 2:
                        pes, pchunks = self.q.pop(0)
                        for mb, ec in pchunks:
                            nc.tensor.matmul(
                                self.po[0:65, :],
                                lhsT=v2[:, mb * 130 + 65 * self.h:mb * 130 + 65 * self.h + 65],
                                rhs=pes[:, ec:ec + 512],
                                start=(mb == 0), stop=(mb == MB - 1),
                            )
                    self.q.append((es, chunks))
                    self.done += w

            def finish(self):
                self.emit_slots(SLOTS)
                pend[0] = (self.nb, self.h, self.po, self.q)

        # ---- prologue: sliced x load, K/V proj, group (0,0) interleaved ----
        g00 = Group(0, 0)
        cast_eng = [nc.vector.tensor_copy, nc.vector.tensor_copy,
                    nc.vector.tensor_copy, nc.gpsimd.tensor_copy]
        for s in range(NB):
            ns = slice(s * 512, (s + 1) * 512)
            stg = []
            for cc in range(CC):
                t = stage.tile([128, 512], F32, tag="xs", name="xs")
                nc.sync.dma_start(out=t[:], in_=xT[cc * 128:(cc + 1) * 128, ns])
                stg.append(t)
            for cc in range(CC):
                cast_eng[cc](out=xtb[cc][:, ns], in_=stg[cc][:])
            emit_k(s)
            if s == 0:
                emit_q(0)
            emit_v4(s)
            g00.emit_slots(min(2 * s + 1, SLOTS))
        g00.finish()

        # ---- steady state ----
        emit_q(1)
        Group(0, 1).finish()
        for nb in range(1, NB):
            Group(nb, 0).finish()
            if nb + 1 < NB:
                emit_q(nb + 1)
            Group(nb, 1).finish()
        flush_pend()
        emit_proj(NB - 1)

    nc.compile()
    return nc


_NC_CACHE = None
LAST_EXEC_NS = None


def kernel(x, w_qkv, w_proj, b_proj):
    global _NC_CACHE, LAST_EXEC_NS
    x = np.ascontiguousarray(np.asarray(x, dtype=np.float32))
    w_qkv = np.asarray(w_qkv, dtype=np.float32)
    w_proj = np.asarray(w_proj, dtype=np.float32)
    b_proj = np.asarray(b_proj, dtype=np.float32)
    B = x.shape[0]

    if _NC_CACHE is None:
        _NC_CACHE = build_nc()
    nc = _NC_CACHE

    xTs = [np.ascontiguousarray(x[b].T) for b in range(B)]
    in_maps = []
    for c in range(8):
        b, hp = c // 4, c % 4
        qr = w_qkv[2 * hp * 64:2 * hp * 64 + 128]
        kr = w_qkv[512 + 2 * hp * 64:512 + 2 * hp * 64 + 128]
        vr = w_qkv[1024 + 2 * hp * 64:1024 + 2 * hp * 64 + 128]
        wqkvT = np.ascontiguousarray(np.concatenate([qr, kr, vr], 0).T)
        wpT = np.ascontiguousarray(w_proj[:, hp * 128:(hp + 1) * 128].T)
        in_maps.append({"xT": xTs[b], "wqkvT": wqkvT, "wpT": wpT})

    res = run_bass_kernel_spmd(
        nc,
        in_maps,
        core_ids=list(range(8)),
        trace=bool(int(os.environ.get("ATTN_TRACE", "0"))),
    )
    LAST_EXEC_NS = res.exec_time_ns

    out = np.zeros((B, N, DIM), np.float32)
    for b in range(B):
        acc = res.results[4 * b]["out"].copy()
        for c in range(4 * b + 1, 4 * b + 4):
            acc += res.results[c]["out"]
        out[b] = acc.T + b_proj
    return out
